# revision 15
# baseline (speedup 1.0000x reference)
"""Trainium2 Bass kernel for nn_BertSelfAttention_10110353015430.

Sharding: Megatron-style tensor parallel over heads. 16 heads / 8 cores =
2 heads per core. Each core computes QKV projection for its 2 heads (both
batches), RoPE, full attention for its heads, and a partial o-projection
(its 128 columns of the 1024-wide contraction). Host sums the 8 partials.

Self-contained: hardcodes all shapes; no sibling imports.
"""

import os
from contextlib import ExitStack

import numpy as np

import concourse.bass as bass
import concourse.mybir as mybir
import concourse.tile as tile
from concourse import bacc, bass_utils
from concourse.bass import ds, ts
from concourse.masks import make_identity

B, L, D = 2, 2048, 1024
H, HD = 16, 64
NCORES = 8
HLOC = H // NCORES          # 2 heads per core
NT = B * L                  # 4096 tokens, laid out [b0 | b1]
F32 = mybir.dt.float32

# matmul dtype: float32r = full-rate fp32 on the PE (vs 4 cyc/row for fp32)
_MM_DT_NAME = os.environ.get("KERNEL_MM_DT", "float32r")
MM_DT = getattr(mybir.dt, _MM_DT_NAME)


def build_body(tc, ins, outs):
    """Per-core program. ins/outs: dicts of DRAM APs.

    ins:
      xT    [1024, 4096]  X^T, tokens = [batch0(2048) | batch1(2048)]
      wqkvT [1024, 384]   cols: q-feats(128) | k-feats(128) | v-feats(128)
      woT   [128, 1024]   rows = this core's 128 attn cols, cols = out dim
      rc    [128, 4096]   RoPE cos table: rc[p, t] = cos[t % L, (p % 64)//2]
      rs    [128, 4096]   RoPE signed sin: -sin on even hd dims, +sin on odd
    outs:
      out   [4096, 1024]  partial o-projection

    RoPE identity used (all contiguous APs; pair-swap via stream_shuffle):
      rot(x) = x * rc + swap_pairs(x) * rs
    """
    nc = tc.nc
    xT, wqkvT, woT = ins["xT"], ins["wqkvT"], ins["woT"]
    rc, rs = ins["rc"], ins["rs"]
    if xT.dtype != MM_DT:
        xT, wqkvT, woT = xT.bitcast(MM_DT), wqkvT.bitcast(MM_DT), woT.bitcast(MM_DT)
        rc, rs = rc.bitcast(MM_DT), rs.bitcast(MM_DT)
    outp = outs["out"]
    swap_mask = [j + 1 if j % 2 == 0 else j - 1 for j in range(32)]

    with ExitStack() as ctx:
        sb = ctx.enter_context(tc.tile_pool(name="sb", bufs=1))
        xp = ctx.enter_context(tc.tile_pool(name="xp", bufs=4))

        # ---- persistent tiles ----
        w_sb = sb.tile([128, 8, 384], MM_DT, tag="w")
        nc.sync.dma_start(w_sb, wqkvT.rearrange("(c p) f -> p c f", p=128))
        woT_sb = sb.tile([128, 1024], MM_DT, tag="wo")
        nc.sync.dma_start(woT_sb, woT)
        rc_sb = sb.tile([128, NT], MM_DT, tag="rc")
        nc.sync.dma_start(rc_sb, rc)
        rs_sb = sb.tile([128, NT], MM_DT, tag="rs")
        nc.sync.dma_start(rs_sb, rs)

        ident = sb.tile([128, 128], F32, tag="id")
        make_identity(nc, ident)

        # Q^T | K^T (staged pre-RoPE, rotated in place),
        # partitions = [h0 hd(64) | h1 hd(64)]
        qkt = sb.tile([128, 2, NT], MM_DT, tag="qkt")
        yt = sb.tile([128, NT], MM_DT, tag="yt")          # RoPE swap temp
        vt0 = sb.tile([128, NT], F32, tag="vt0")        # V^T (pre-transpose)
        # V per 128-token tile, per head, with ones column at free idx 64
        vall = sb.tile([128, 32, HLOC, 65], MM_DT, tag="vall")
        nc.vector.memset(vall.bitcast(F32), 1.0)
        # attention out O^T, partitions = [h0(64) | h1(64)], free = tokens
        ot = sb.tile([128, NT], MM_DT, tag="ot")

        # ---- phase 1: QKV projection + RoPE + V transpose ----
        with tc.tile_pool(name="pq", bufs=2, space="PSUM") as pq:
            for tch in range(8):              # 512-token chunks
                sl = ds(tch * 512, 512)
                q_ps = pq.tile([128, 512], F32, tag="q")
                k_ps = pq.tile([128, 512], F32, tag="k")
                v_ps = pq.tile([128, 512], F32, tag="v")
                for dch in range(8):          # contraction chunks of 128
                    xb = xp.tile([128, 512], MM_DT, tag="xb")
                    nc.sync.dma_start(
                        xb, xT[ds(dch * 128, 128), sl]
                    )
                    st, sp = dch == 0, dch == 7
                    nc.tensor.matmul(
                        q_ps, w_sb[:, dch, 0:128], xb, start=st, stop=sp
                    )
                    nc.tensor.matmul(
                        k_ps, w_sb[:, dch, 128:256], xb, start=st, stop=sp
                    )
                    nc.tensor.matmul(
                        v_ps, w_sb[:, dch, 256:384], xb, start=st, stop=sp
                    )
                nc.vector.tensor_copy(qkt[:, 0, sl], q_ps)
                nc.vector.tensor_copy(qkt[:, 1, sl], k_ps)
                nc.vector.tensor_copy(vt0[:, sl], v_ps)
                # transpose V^T -> V for the 4 token-tiles of this chunk
                # (both heads at once: out = [128 tokens, 128 vfeats])
                for i in range(4):
                    tt = tch * 4 + i
                    vtp = pq.tile([128, 128], F32, tag="vt")
                    nc.tensor.transpose(
                        vtp, vt0[:, ds(tt * 128, 128)], ident
                    )
                    nc.vector.tensor_copy(
                        vall[:, tt, :, 0:64], vtp.rearrange("p (h d) -> p h d", h=2)
                    )

        # ---- phase 1b: RoPE on q and k, in place ----
        # rot(x) = x*rc + swap_pairs(x)*rs
        for si in range(2):
            qk = qkt[:, si, :]                  # [128, 4096]
            nc.vector.stream_shuffle(yt.bitcast(F32), qk.bitcast(F32), swap_mask)
            nc.vector.tensor_mul(qk, qk, rc_sb)
            nc.vector.tensor_mul(yt, yt, rs_sb)
            nc.vector.tensor_add(qk, qk, yt)

        # ---- phase 2: attention ----
        ptp = ctx.enter_context(tc.tile_pool(name="ptp", bufs=3))
        dnp = ctx.enter_context(tc.tile_pool(name="dnp", bufs=2))
        with tc.tile_pool(name="pa", bufs=2, space="PSUM") as pa:
            for b in range(B):
                for h in range(HLOC):
                    qth = qkt[ds(h * 64, 64), 0, ds(b * L, L)]   # [64, 2048]
                    kth = qkt[ds(h * 64, 64), 1, ds(b * L, L)]
                    for qg in range(2):                          # 1024-q groups
                        ov = pa.tile([65, 1024], F32, tag="ov")
                        for kt in range(16):                     # 128-k tiles
                            s_ps = pa.tile([128, 1024], F32, tag="s")
                            for qi in range(2):
                                nc.tensor.matmul(
                                    s_ps[:, ds(qi * 512, 512)],
                                    kth[:, ds(kt * 128, 128)],
                                    qth[:, ds(qg * 1024 + qi * 512, 512)],
                                    start=True,
                                    stop=True,
                                )
                            pt = ptp.tile([128, 1024], MM_DT, tag="pt")
                            nc.scalar.activation(
                                pt, s_ps, mybir.ActivationFunctionType.Exp,
                                scale=0.125,
                            )
                            vtile = vall[:, b * 16 + kt, h, :]   # [128, 65]
                            for qi in range(2):
                                nc.tensor.matmul(
                                    ov[:, ds(qi * 512, 512)],
                                    vtile,
                                    pt[:, ds(qi * 512, 512)],
                                    start=(kt == 0),
                                    stop=(kt == 15),
                                )
                        den = dnp.tile([1, 1024], F32, tag="den")
                        nc.vector.reciprocal(den, ov[64:65, :])
                        denb = dnp.tile([64, 1024], F32, tag="denb")
                        nc.gpsimd.partition_broadcast(denb, den)
                        nc.vector.tensor_mul(
                            ot[ds(h * 64, 64), ds(b * L + qg * 1024, 1024)],
                            ov[0:64, :],
                            denb,
                        )

        # ---- phase 3: partial o-projection ----
        obp = ctx.enter_context(tc.tile_pool(name="obp", bufs=3))
        with tc.tile_pool(name="po", bufs=2, space="PSUM") as po:
            for tt in range(32):
                for ni in range(2):
                    op_ps = po.tile([128, 512], F32, tag="op")
                    nc.tensor.matmul(
                        op_ps,
                        ot[:, ds(tt * 128, 128)],
                        woT_sb[:, ds(ni * 512, 512)],
                        start=True,
                        stop=True,
                    )
                    ob = obp.tile([128, 512], F32, tag="ob")
                    nc.vector.tensor_copy(ob, op_ps)
                    nc.sync.dma_start(
                        outp[ds(tt * 128, 128), ds(ni * 512, 512)], ob
                    )


def _prep_inputs(hidden_states, w_qkv, w_o, freqs_cos, freqs_sin):
    """Host-side prep: transpose X, slice per-core weights, pack cos/sin."""
    x = np.ascontiguousarray(
        np.asarray(hidden_states, dtype=np.float32).reshape(NT, D).T
    )  # [1024, 4096]
    w_qkv = np.asarray(w_qkv, dtype=np.float32)
    w_o = np.asarray(w_o, dtype=np.float32)
    cosT = np.asarray(freqs_cos, dtype=np.float32).T     # [32, 2048]
    sinT = np.asarray(freqs_sin, dtype=np.float32).T
    # RoPE tables: partition p -> head p//64, hd dim d = p%64, pair j = d//2
    # rc[p] = cos[j], rs[p] = (-1 if d even else +1) * sin[j]
    j_of_p = (np.arange(128) % 64) // 2                  # [128]
    sign = np.where(np.arange(128) % 2 == 0, -1.0, 1.0).astype(np.float32)
    rc1 = cosT[j_of_p]                                   # [128, 2048]
    rs1 = sinT[j_of_p] * sign[:, None]
    rc = np.ascontiguousarray(np.tile(rc1, (1, B)))      # [128, 4096]
    rs = np.ascontiguousarray(np.tile(rs1, (1, B)))

    in_maps = []
    for c in range(NCORES):
        rows = slice(c * HLOC * HD, (c + 1) * HLOC * HD)   # 128 feat rows
        wq = w_qkv[0 * D : 1 * D][rows]                    # [128, 1024]
        wk = w_qkv[1 * D : 2 * D][rows]
        wv = w_qkv[2 * D : 3 * D][rows]
        wqkvT = np.ascontiguousarray(
            np.concatenate([wq, wk, wv], axis=0).T         # [1024, 384]
        )
        woT = np.ascontiguousarray(w_o[:, rows].T)         # [128, 1024]
        in_maps.append({"xT": x, "wqkvT": wqkvT, "woT": woT, "rc": rc, "rs": rs})
    return in_maps


_CACHE = {}


def _get_module():
    if "nc" in _CACHE:
        return _CACHE["nc"]
    nc = bacc.Bacc(
        "TRN2",
        target_bir_lowering=False,
        debug=False,
        enable_asserts=True,
        num_devices=NCORES,
    )
    ins = {
        "xT": nc.dram_tensor("xT", [D, NT], F32, kind="ExternalInput").ap(),
        "wqkvT": nc.dram_tensor("wqkvT", [D, 384], F32, kind="ExternalInput").ap(),
        "woT": nc.dram_tensor("woT", [128, D], F32, kind="ExternalInput").ap(),
        "rc": nc.dram_tensor("rc", [128, NT], F32, kind="ExternalInput").ap(),
        "rs": nc.dram_tensor("rs", [128, NT], F32, kind="ExternalInput").ap(),
    }
    outs = {
        "out": nc.dram_tensor("out", [NT, D], F32, kind="ExternalOutput").ap(),
    }
    with tile.TileContext(nc) as tc:
        build_body(tc, ins, outs)
    nc.compile()
    _CACHE["nc"] = nc
    return nc


LAST_RESULTS = None


def kernel(hidden_states, w_qkv, w_o, freqs_cos, freqs_sin, mask=None):
    global LAST_RESULTS
    nc = _get_module()
    in_maps = _prep_inputs(hidden_states, w_qkv, w_o, freqs_cos, freqs_sin)
    trace = os.environ.get("KERNEL_TRACE", "0") == "1"
    res = bass_utils.run_bass_kernel_spmd(
        nc, in_maps, core_ids=list(range(NCORES)), trace=trace
    )
    LAST_RESULTS = res
    acc = res.results[0]["out"].astype(np.float64)
    for c in range(1, NCORES):
        acc += res.results[c]["out"]
    return acc.astype(np.float32).reshape(B, L, D)


# revision 17
# speedup vs baseline: 106.2522x; 106.2522x over previous
"""Trainium2 Bass kernel for nn_BertSelfAttention_10110353015430.

Sharding: Megatron-style tensor parallel over heads. 16 heads / 8 cores =
2 heads per core. Each core computes QKV projection for its 2 heads (both
batches), RoPE, full attention for its heads, and a partial o-projection
(its 128 columns of the 1024-wide contraction). Host sums the 8 partials.

Self-contained: hardcodes all shapes; no sibling imports.
"""

import os
from contextlib import ExitStack

import numpy as np

import concourse.bass as bass
import concourse.mybir as mybir
import concourse.tile as tile
from concourse import bacc, bass_utils
from concourse.bass import ds, ts
from concourse.masks import make_identity

B, L, D = 2, 2048, 1024
H, HD = 16, 64
NCORES = 8
HLOC = H // NCORES          # 2 heads per core
NT = B * L                  # 4096 tokens, laid out [b0 | b1]
F32 = mybir.dt.float32

# matmul dtype: float32r = full-rate fp32 on the PE (vs 4 cyc/row for fp32)
_MM_DT_NAME = os.environ.get("KERNEL_MM_DT", "float32r")
MM_DT = getattr(mybir.dt, _MM_DT_NAME)


def build_body(tc, ins, outs):
    """Per-core program. ins/outs: dicts of DRAM APs.

    ins:
      xT    [1024, 4096]  X^T, tokens = [batch0(2048) | batch1(2048)]
      wqkvT [1024, 384]   cols: q-feats(128) | k-feats(128) | v-feats(128)
      woT   [128, 1024]   rows = this core's 128 attn cols, cols = out dim
      rc    [128, 4096]   RoPE cos table: rc[p, t] = cos[t % L, (p % 64)//2]
      rs    [128, 4096]   RoPE signed sin: -sin on even hd dims, +sin on odd
    outs:
      out   [4096, 1024]  partial o-projection

    RoPE identity used (all contiguous APs; pair-swap via stream_shuffle):
      rot(x) = x * rc + swap_pairs(x) * rs
    """
    nc = tc.nc
    xT, wqkvT, woT = ins["xT"], ins["wqkvT"], ins["woT"]
    rc, rs = ins["rc"], ins["rs"]
    if xT.dtype != MM_DT:
        xT, wqkvT, woT = xT.bitcast(MM_DT), wqkvT.bitcast(MM_DT), woT.bitcast(MM_DT)
        rc, rs = rc.bitcast(MM_DT), rs.bitcast(MM_DT)
    outp = outs["out"]
    swap_mask = [j + 1 if j % 2 == 0 else j - 1 for j in range(32)]

    with ExitStack() as ctx:
        sb = ctx.enter_context(tc.tile_pool(name="sb", bufs=1))
        xp = ctx.enter_context(tc.tile_pool(name="xp", bufs=4))

        # ---- persistent tiles ----
        w_sb = sb.tile([128, 8, 384], MM_DT, tag="w")
        nc.sync.dma_start(w_sb, wqkvT.rearrange("(c p) f -> p c f", p=128))
        woT_sb = sb.tile([128, 1024], MM_DT, tag="wo")
        nc.sync.dma_start(woT_sb, woT)
        rc_sb = sb.tile([128, NT], MM_DT, tag="rc")
        nc.sync.dma_start(rc_sb, rc)
        rs_sb = sb.tile([128, NT], MM_DT, tag="rs")
        nc.sync.dma_start(rs_sb, rs)

        ident = sb.tile([128, 128], F32, tag="id")
        make_identity(nc, ident)

        # Q^T | K^T (staged pre-RoPE, rotated in place),
        # partitions = [h0 hd(64) | h1 hd(64)]
        qkt = sb.tile([128, 2, NT], MM_DT, tag="qkt")
        yt = sb.tile([128, NT], MM_DT, tag="yt")          # RoPE swap temp
        vt0 = sb.tile([128, NT], F32, tag="vt0")        # V^T (pre-transpose)
        # V per 128-token tile, per head, with ones column at free idx 64
        vall = sb.tile([128, 32, HLOC, 65], MM_DT, tag="vall")
        nc.vector.memset(vall.bitcast(F32), 1.0)
        # attention out O^T, partitions = [h0(64) | h1(64)], free = tokens
        ot = sb.tile([128, NT], MM_DT, tag="ot")

        # ---- phase 1: QKV projection + RoPE + V transpose ----
        with tc.tile_pool(name="pq", bufs=2, space="PSUM") as pq:
            for tch in range(8):              # 512-token chunks
                sl = ds(tch * 512, 512)
                q_ps = pq.tile([128, 512], F32, tag="q")
                k_ps = pq.tile([128, 512], F32, tag="k")
                v_ps = pq.tile([128, 512], F32, tag="v")
                for dch in range(8):          # contraction chunks of 128
                    xb = xp.tile([128, 512], MM_DT, tag="xb")
                    nc.sync.dma_start(
                        xb, xT[ds(dch * 128, 128), sl]
                    )
                    st, sp = dch == 0, dch == 7
                    nc.tensor.matmul(
                        q_ps, w_sb[:, dch, 0:128], xb, start=st, stop=sp
                    )
                    nc.tensor.matmul(
                        k_ps, w_sb[:, dch, 128:256], xb, start=st, stop=sp
                    )
                    nc.tensor.matmul(
                        v_ps, w_sb[:, dch, 256:384], xb, start=st, stop=sp
                    )
                nc.vector.tensor_copy(qkt[:, 0, sl], q_ps)
                nc.vector.tensor_copy(qkt[:, 1, sl], k_ps)
                nc.vector.tensor_copy(vt0[:, sl], v_ps)
                # transpose V^T -> V for the 4 token-tiles of this chunk
                # (both heads at once: out = [128 tokens, 128 vfeats])
                for i in range(4):
                    tt = tch * 4 + i
                    vtp = pq.tile([128, 128], F32, tag="vt")
                    nc.tensor.transpose(
                        vtp, vt0[:, ds(tt * 128, 128)], ident
                    )
                    nc.vector.tensor_copy(
                        vall[:, tt, :, 0:64], vtp.rearrange("p (h d) -> p h d", h=2)
                    )

        # ---- phase 1b: RoPE on q and k, in place ----
        # rot(x) = x*rc + swap_pairs(x)*rs
        for si in range(2):
            qk = qkt[:, si, :]                  # [128, 4096]
            nc.vector.stream_shuffle(yt.bitcast(F32), qk.bitcast(F32), swap_mask)
            nc.vector.tensor_mul(qk, qk, rc_sb)
            nc.vector.tensor_mul(yt, yt, rs_sb)
            nc.vector.tensor_add(qk, qk, yt)

        # ---- phase 2: attention ----
        ptp = ctx.enter_context(tc.tile_pool(name="ptp", bufs=3))
        dnp = ctx.enter_context(tc.tile_pool(name="dnp", bufs=2))
        with tc.tile_pool(name="pa", bufs=2, space="PSUM") as pa:
            for b in range(B):
                for h in range(HLOC):
                    qth = qkt[ds(h * 64, 64), 0, ds(b * L, L)]   # [64, 2048]
                    kth = qkt[ds(h * 64, 64), 1, ds(b * L, L)]
                    for qg in range(2):                          # 1024-q groups
                        ov = pa.tile([65, 1024], F32, tag="ov")
                        for kt in range(16):                     # 128-k tiles
                            s_ps = pa.tile([128, 1024], F32, tag="s")
                            for qi in range(2):
                                nc.tensor.matmul(
                                    s_ps[:, ds(qi * 512, 512)],
                                    kth[:, ds(kt * 128, 128)],
                                    qth[:, ds(qg * 1024 + qi * 512, 512)],
                                    start=True,
                                    stop=True,
                                )
                            pt = ptp.tile([128, 1024], MM_DT, tag="pt")
                            nc.scalar.activation(
                                pt, s_ps, mybir.ActivationFunctionType.Exp,
                                scale=0.125,
                            )
                            vtile = vall[:, b * 16 + kt, h, :]   # [128, 65]
                            for qi in range(2):
                                nc.tensor.matmul(
                                    ov[:, ds(qi * 512, 512)],
                                    vtile,
                                    pt[:, ds(qi * 512, 512)],
                                    start=(kt == 0),
                                    stop=(kt == 15),
                                )
                        den = dnp.tile([1, 1024], F32, tag="den")
                        nc.vector.reciprocal(den, ov[64:65, :])
                        denb = dnp.tile([64, 1024], F32, tag="denb")
                        nc.gpsimd.partition_broadcast(denb, den)
                        nc.vector.tensor_mul(
                            ot[ds(h * 64, 64), ds(b * L + qg * 1024, 1024)],
                            ov[0:64, :],
                            denb,
                        )

        # ---- phase 3: partial o-projection ----
        obp = ctx.enter_context(tc.tile_pool(name="obp", bufs=3))
        with tc.tile_pool(name="po", bufs=2, space="PSUM") as po:
            for tt in range(32):
                for ni in range(2):
                    op_ps = po.tile([128, 512], F32, tag="op")
                    nc.tensor.matmul(
                        op_ps,
                        ot[:, ds(tt * 128, 128)],
                        woT_sb[:, ds(ni * 512, 512)],
                        start=True,
                        stop=True,
                    )
                    ob = obp.tile([128, 512], F32, tag="ob")
                    nc.vector.tensor_copy(ob, op_ps)
                    nc.sync.dma_start(
                        outp[ds(tt * 128, 128), ds(ni * 512, 512)], ob
                    )


def _prep_inputs(hidden_states, w_qkv, w_o, freqs_cos, freqs_sin):
    """Host-side prep: transpose X, slice per-core weights, pack cos/sin."""
    x = np.ascontiguousarray(
        np.asarray(hidden_states, dtype=np.float32).reshape(NT, D).T
    )  # [1024, 4096]
    w_qkv = np.asarray(w_qkv, dtype=np.float32)
    w_o = np.asarray(w_o, dtype=np.float32)
    cosT = np.asarray(freqs_cos, dtype=np.float32).T     # [32, 2048]
    sinT = np.asarray(freqs_sin, dtype=np.float32).T
    # RoPE tables: partition p -> head p//64, hd dim d = p%64, pair j = d//2
    # rc[p] = cos[j], rs[p] = (-1 if d even else +1) * sin[j]
    j_of_p = (np.arange(128) % 64) // 2                  # [128]
    sign = np.where(np.arange(128) % 2 == 0, -1.0, 1.0).astype(np.float32)
    rc1 = cosT[j_of_p]                                   # [128, 2048]
    rs1 = sinT[j_of_p] * sign[:, None]
    rc = np.ascontiguousarray(np.tile(rc1, (1, B)))      # [128, 4096]
    rs = np.ascontiguousarray(np.tile(rs1, (1, B)))

    in_maps = []
    for c in range(NCORES):
        rows = slice(c * HLOC * HD, (c + 1) * HLOC * HD)   # 128 feat rows
        wq = w_qkv[0 * D : 1 * D][rows]                    # [128, 1024]
        wk = w_qkv[1 * D : 2 * D][rows]
        wv = w_qkv[2 * D : 3 * D][rows]
        wqkvT = np.ascontiguousarray(
            np.concatenate([wq, wk, wv], axis=0).T         # [1024, 384]
        )
        woT = np.ascontiguousarray(w_o[:, rows].T)         # [128, 1024]
        in_maps.append({"xT": x, "wqkvT": wqkvT, "woT": woT, "rc": rc, "rs": rs})
    return in_maps


_CACHE = {}


def _get_module():
    if "nc" in _CACHE:
        return _CACHE["nc"]
    nc = bacc.Bacc(
        "TRN2",
        target_bir_lowering=False,
        debug=False,
        enable_asserts=True,
        num_devices=NCORES,
    )
    ins = {
        "xT": nc.dram_tensor("xT", [D, NT], F32, kind="ExternalInput").ap(),
        "wqkvT": nc.dram_tensor("wqkvT", [D, 384], F32, kind="ExternalInput").ap(),
        "woT": nc.dram_tensor("woT", [128, D], F32, kind="ExternalInput").ap(),
        "rc": nc.dram_tensor("rc", [128, NT], F32, kind="ExternalInput").ap(),
        "rs": nc.dram_tensor("rs", [128, NT], F32, kind="ExternalInput").ap(),
    }
    outs = {
        "out": nc.dram_tensor("out", [NT, D], F32, kind="ExternalOutput").ap(),
    }
    with tile.TileContext(nc) as tc:
        build_body(tc, ins, outs)
    nc.compile()
    _CACHE["nc"] = nc
    return nc


def _get_runner():
    """Compiled SPMD runner with device-resident inputs (mirrors
    bass2jax.run_bass_via_pjrt, but caches the jitted callable and keeps
    inputs on device so repeat calls measure pure device execution)."""
    if "runner" in _CACHE:
        return _CACHE["runner"]
    import jax
    import jax.numpy as jnp
    from jax.experimental.shard_map import shard_map
    from jax.sharding import Mesh, NamedSharding, PartitionSpec

    from concourse import bass2jax, mybir as _mybir

    nc = _get_module()
    bass2jax.install_neuronx_cc_hook()

    part_name = nc.partition_id_tensor.name if nc.partition_id_tensor else None
    in_names, out_names, out_avals = [], [], []
    for alloc in nc.m.functions[0].allocations:
        if not isinstance(_mybir.MemoryLocationSet, type) or not isinstance(
            alloc, _mybir.MemoryLocationSet
        ):
            continue
        name = alloc.memorylocations[0].name
        if alloc.kind == "ExternalInput":
            if name != part_name:
                in_names.append(name)
        elif alloc.kind == "ExternalOutput":
            shape = tuple(alloc.tensor_shape)
            dtype = _mybir.dt.np(alloc.dtype)
            out_names.append(name)
            out_avals.append(jax.core.ShapedArray(shape, dtype))
    n_params = len(in_names)
    all_in_names = in_names + out_names
    if part_name is not None:
        all_in_names = all_in_names + [part_name]

    def _body(*args):
        operands = list(args)
        if part_name is not None:
            operands.append(bass2jax.partition_id_tensor())
        outs = bass2jax._bass_exec_p.bind(
            *operands,
            out_avals=tuple(out_avals),
            in_names=tuple(all_in_names),
            out_names=tuple(out_names),
            lowering_input_output_aliases=(),
            sim_require_finite=True,
            sim_require_nnan=True,
            nc=nc,
        )
        return tuple(outs)

    devices = jax.devices()[:NCORES]
    mesh = Mesh(np.asarray(devices), ("core",))
    spec = NamedSharding(mesh, PartitionSpec("core"))
    n_outs = len(out_avals)
    donate = tuple(range(n_params, n_params + n_outs))
    sharded = jax.jit(
        shard_map(
            _body,
            mesh=mesh,
            in_specs=(PartitionSpec("core"),) * (n_params + n_outs),
            out_specs=(PartitionSpec("core"),) * n_outs,
            check_rep=False,
        ),
        donate_argnums=donate,
        keep_unused=True,
    )

    zero_shapes = [
        (NCORES * a.shape[0], *a.shape[1:]) for a in out_avals
    ]
    zeros_fn = jax.jit(
        lambda: tuple(
            jnp.zeros(s, a.dtype) for s, a in zip(zero_shapes, out_avals)
        ),
        out_shardings=(spec,) * n_outs,
    )

    runner = {
        "sharded": sharded,
        "zeros_fn": zeros_fn,
        "in_names": in_names,
        "out_names": out_names,
        "out_avals": out_avals,
        "spec": spec,
        "jax": jax,
    }
    _CACHE["runner"] = runner
    return runner


def _device_inputs(in_maps):
    r = _get_runner()
    jax = r["jax"]
    concat = [
        np.concatenate([in_maps[c][name] for c in range(NCORES)], axis=0)
        for name in r["in_names"]
    ]
    return [jax.device_put(a, r["spec"]) for a in concat]


def _run_once(dev_inputs):
    r = _get_runner()
    zeros = r["zeros_fn"]()
    outs = r["sharded"](*dev_inputs, *zeros)
    r["jax"].block_until_ready(outs)
    return outs


def bench(dev_inputs, iters=3):
    import time as _time

    _run_once(dev_inputs)  # warm
    times = []
    for _ in range(iters):
        r = _get_runner()
        zeros = r["zeros_fn"]()
        r["jax"].block_until_ready(zeros)
        t0 = _time.perf_counter()
        outs = r["sharded"](*dev_inputs, *zeros)
        r["jax"].block_until_ready(outs)
        times.append(_time.perf_counter() - t0)
    return min(times)


def kernel(hidden_states, w_qkv, w_o, freqs_cos, freqs_sin, mask=None):
    in_maps = _prep_inputs(hidden_states, w_qkv, w_o, freqs_cos, freqs_sin)
    dev_inputs = _device_inputs(in_maps)
    outs = _run_once(dev_inputs)
    r = _get_runner()
    # outs[i] is the global [8*4096, 1024] array; per-core shards on axis 0
    out_g = np.asarray(outs[0]).reshape(NCORES, NT, D)
    acc = out_g.astype(np.float64).sum(axis=0)
    return acc.astype(np.float32).reshape(B, L, D)
